# revision 11
# baseline (speedup 1.0000x reference)
"""Chamfer loss kernel for Trainium2 (8 NeuronCores).

Problem: x, y: [4, 3, 8192] f32.  d2[b,n,m] = ||x[b,:,n] - y[b,:,m]||^2.
out = mean_n(min_m d2) + mean_m(min_n d2)  (scalar f32).

Sharding: core c -> batch c//2, point-half c%2.  Each core runs two
symmetric passes (x-side and y-side row-mins over the full opposing
cloud), so every core's outputs are final mins for a disjoint set of
points and no cross-core reduction is needed.

Device math: one K=15 bf16 matmul per (n-tile, m-block) produces
psum[n,m] = y^2[m] - 2*x.y  (to ~2^-18 relative) via hi/lo split rows:

  k 0..2:   W=-2*xh_d  R=yh_d        k 9..11:  W=1  R=hi(y_d^2)
  k 3..5:   W=-2*xl_d  R=yh_d        k 12..14: W=1  R=lo(y_d^2)
  k 6..8:   W=-2*xh_d  R=yl_d

bf16 products are exact in f32 PSUM; only the xl*yl term (~2^-18) is
dropped.  fp32 matmuls would be ~5x slower on the PE (hi/lo double
pass at half stream rate).

Row-min over m is extracted with a custom fused DVE op
(min(in0,in1) + min-accumulate) that consumes one PSUM tile and one
ScalarE-copied SBUF tile per instruction.  The per-point +x^2[n] and
final means are O(N) host post-processing, as is building the split
rows (host numpy, O(N)).
"""

import sys

if '/opt/trn_rl_repo' not in sys.path:
    sys.path.insert(0, '/opt/trn_rl_repo')

import ml_dtypes
import numpy as np

import concourse.bacc as bacc
import concourse.mybir as mybir
import concourse.tile as tile
from concourse.bass_utils import run_bass_kernel_spmd

import concourse.dve_ops as dve_ops_mod
from concourse.dve_ops import DveOp
from concourse.dve_spec import (Spec, Src0, Src1, C0, minn, lower, AluOp,
                                _has_src1)
from concourse.dve_uop import DveOpSpec

F32 = mybir.dt.float32
BF16 = mybir.dt.bfloat16
NPBF16 = ml_dtypes.bfloat16
BIG = 3.0e38

B = 4
C = 3
K = 15        # split-K augmented contraction dim
NPTS = 8192   # points per cloud
NSHARD = NPTS // 2  # points handled per core per side
N_CORES = 8


def _ref_min2(in0, in1, c0, c1, c2):
    b = np.minimum(in0.astype(np.float32), in1.astype(np.float32))
    return b, np.minimum(
        np.asarray(c0, np.float32).reshape(-1, 1) if np.ndim(c0) else np.float32(c0),
        b.reshape(b.shape[0], -1).min(axis=-1, keepdims=True))


def register_min2():
    """Custom DVE op: out = min(in0, in1); accum_out = min(s0, min(out)).

    The standard-ISA TENSOR_TENSOR_REDUCE opcode is not supported by the
    runtime here, but custom-DVE ops ship their own uop table with the NEFF.
    This fused op consumes two 512-wide tiles per instruction (one PSUM, one
    SBUF), which is what keeps the DVE at ~0.75 cycles per reduced column."""
    name = "CHAMFER_MIN2_REDUCE"
    if name in dve_ops_mod._SUB_OPCODE_FOR_NAME:
        return next(op for op in dve_ops_mod.OPS if op.name == name)
    spec = Spec(body=minn(Src0, Src1), accum=AluOp.MIN, accum_init=C0,
                reference=_ref_min2)
    row = dve_ops_mod._CUSTOM_DVE_ROW_BASE + len(dve_ops_mod.OPS)
    dve_ops_mod._SUB_OPCODE_FOR_NAME[name] = row
    shas = {}
    for ver in ("v3", "v4"):
        uops = lower(spec, ver=ver)
        shas[ver] = DveOpSpec(name=name, opcode=row, uops=uops,
                              rd1_en=_has_src1(spec)).sha(ver)
    op = DveOp(name, spec, subdim=False, uops_sha=shas)
    dve_ops_mod.OPS.append(op)
    dve_ops_mod.CUSTOM_DVE_SPECS[name] = spec
    return op


MIN2 = register_min2()


def _emit_load(nc, pools, w_dram, r_dram, tag):
    """Chunked input DMAs so the first matmuls can start early."""
    const_pool = pools["const"]
    W = const_pool.tile([K, NSHARD], BF16, tag=f"W_{tag}")
    for j in range(4):
        s = slice(j * (NSHARD // 4), (j + 1) * (NSHARD // 4))
        nc.sync.dma_start(W[:, s], w_dram[:, s])
    R = const_pool.tile([K, NPTS], BF16, tag=f"R_{tag}")
    for j in range(8):
        s = slice(j * (NPTS // 8), (j + 1) * (NPTS // 8))
        nc.sync.dma_start(R[:, s], r_dram[:, s])
    return W, R


def _emit_pass(nc, tc, pools, W, R, out_dram, tag):
    """One pass: W [K, NSHARD] bf16 weight rows, R [K, NPTS] bf16 rhs rows,
    out [128, NT] f32 row-mins (partition = point % 128, col = point//128).

    Per (n-tile, pair): 4 matmuls fill two 2-bank psum tiles [128, 1024];
    ScalarE copies the second to SBUF; the fused MIN2 DVE op consumes the
    pair and min-accumulates the row-min."""
    NT = NSHARD // 128       # weight tiles
    NP = NPTS // 2048        # pair count (each pair covers 2048 m-columns)

    psum_pool = pools["psum"]
    copy_pool = pools["copy"]
    scratch_pool = pools["scratch"]
    accum_pool = pools["accum"]

    minbuf = pools["const"].tile([128, NT], F32, tag=f"minbuf_{tag}")

    for t in range(NT):
        wslice = W[:, t * 128:(t + 1) * 128]
        accum = accum_pool.tile([128, NP], F32, tag="acc")
        for i in range(NP):
            pa = psum_pool.tile([128, 1024], F32, tag="ps")
            pb = psum_pool.tile([128, 1024], F32, tag="ps")
            base = i * 2048
            nc.tensor.matmul(pa[:, 0:512], wslice,
                             R[:, base:base + 512], start=True, stop=True)
            nc.tensor.matmul(pa[:, 512:1024], wslice,
                             R[:, base + 512:base + 1024],
                             start=True, stop=True)
            nc.tensor.matmul(pb[:, 0:512], wslice,
                             R[:, base + 1024:base + 1536],
                             start=True, stop=True)
            nc.tensor.matmul(pb[:, 512:1024], wslice,
                             R[:, base + 1536:base + 2048],
                             start=True, stop=True)
            cp = copy_pool.tile([128, 1024], F32, tag="cp")
            nc.scalar.copy(cp[:], pb[:])
            scr = scratch_pool.tile([128, 1024], F32, tag="scr")
            nc.vector._custom_dve(MIN2, out=scr[:], in0=pa[:], in1=cp[:],
                                  s0=BIG, accum_out=accum[:, i:i + 1])
        nc.vector.tensor_reduce(minbuf[:, t:t + 1], accum[:],
                                axis=mybir.AxisListType.X,
                                op=mybir.AluOpType.min)

    nc.sync.dma_start(out_dram[:], minbuf[:])


def build_program():
    from contextlib import ExitStack
    nc = bacc.Bacc("TRN2", target_bir_lowering=False, debug=False)
    NT = NSHARD // 128

    wa = nc.dram_tensor("wa", [K, NSHARD], BF16, kind="ExternalInput")
    ra = nc.dram_tensor("ra", [K, NPTS], BF16, kind="ExternalInput")
    wb = nc.dram_tensor("wb", [K, NSHARD], BF16, kind="ExternalInput")
    rb = nc.dram_tensor("rb", [K, NPTS], BF16, kind="ExternalInput")
    minx = nc.dram_tensor("minx", [128, NT], F32, kind="ExternalOutput")
    miny = nc.dram_tensor("miny", [128, NT], F32, kind="ExternalOutput")

    with tile.TileContext(nc) as tc:
        with ExitStack() as ctx:
            pools = {
                "const": ctx.enter_context(tc.tile_pool(name="const", bufs=1)),
                "psum": ctx.enter_context(
                    tc.tile_pool(name="psum", bufs=4, space="PSUM")),
                "copy": ctx.enter_context(tc.tile_pool(name="copy", bufs=3)),
                "scratch": ctx.enter_context(tc.tile_pool(name="scr", bufs=2)),
                "accum": ctx.enter_context(tc.tile_pool(name="acc", bufs=2)),
            }
            # all input loads emitted first: pass-B inputs prefetch during
            # pass A instead of queueing behind pass-A's output DMA
            Wa, Ra = _emit_load(nc, pools, wa, ra, "a")
            Wb, Rb = _emit_load(nc, pools, wb, rb, "b")
            _emit_pass(nc, tc, pools, Wa, Ra, minx, "a")
            _emit_pass(nc, tc, pools, Wb, Rb, miny, "b")
    nc.compile()
    return nc


_cached_nc = None


def _get_nc():
    global _cached_nc
    if _cached_nc is None:
        _cached_nc = build_program()
    return _cached_nc


def _split_w(shard):
    """shard: [3, n] f32 -> [K, n] bf16 weight rows."""
    n = shard.shape[1]
    xh = shard.astype(NPBF16)
    xl = (shard - xh.astype(np.float32)).astype(NPBF16)
    w = np.empty((K, n), NPBF16)
    w[0:3] = (-2.0 * xh.astype(np.float32)).astype(NPBF16)   # exact scale
    w[3:6] = (-2.0 * xl.astype(np.float32)).astype(NPBF16)
    w[6:9] = w[0:3]
    w[9:15] = NPBF16(1.0)
    return w


def _split_r(full):
    """full: [3, m] f32 -> [K, m] bf16 rhs rows."""
    m = full.shape[1]
    yh = full.astype(NPBF16)
    yl = (full - yh.astype(np.float32)).astype(NPBF16)
    sq = (full.astype(np.float32) ** 2)
    sqh = sq.astype(NPBF16)
    sql = (sq - sqh.astype(np.float32)).astype(NPBF16)
    r = np.empty((K, m), NPBF16)
    r[0:3] = yh
    r[3:6] = yh
    r[6:9] = yl
    r[9:12] = sqh
    r[12:15] = sql
    return r


def run_sharded(x, y, trace=False, **kw):
    """Returns (scalar_out, BassKernelResults)."""
    x = np.ascontiguousarray(x, dtype=np.float32)
    y = np.ascontiguousarray(y, dtype=np.float32)
    nc = _get_nc()
    in_maps = []
    for c in range(N_CORES):
        b, h = c // 2, c % 2
        sl = slice(h * NSHARD, (h + 1) * NSHARD)
        in_maps.append({
            "wa": _split_w(x[b, :, sl]),
            "ra": _split_r(y[b]),
            "wb": _split_w(y[b, :, sl]),
            "rb": _split_r(x[b]),
        })
    res = run_bass_kernel_spmd(nc, in_maps, core_ids=list(range(N_CORES)),
                               trace=trace, **kw)

    # Host epilogue: add ||p||^2 for each sharded point, then mean.
    x2 = np.sum(x.astype(np.float64) ** 2, axis=1)  # [B, NPTS]
    y2 = np.sum(y.astype(np.float64) ** 2, axis=1)  # [B, NPTS]
    sx = 0.0
    sy = 0.0
    for c in range(N_CORES):
        b, h = c // 2, c % 2
        sl = slice(h * NSHARD, (h + 1) * NSHARD)
        vx = res.results[c]["minx"].T.reshape(-1).astype(np.float64)
        vy = res.results[c]["miny"].T.reshape(-1).astype(np.float64)
        sx += np.sum(vx + x2[b, sl])
        sy += np.sum(vy + y2[b, sl])
    out = np.float32(sx / (B * NPTS) + sy / (B * NPTS))
    return out, res


def kernel(x, y):
    out, _ = run_sharded(x, y, trace=False)
    return out
